# revision 7
# baseline (speedup 1.0000x reference)
"""Trainium2 Bass kernel for nn_ButterflyNetwork (self-contained).

Strategy:
- Pure data parallelism: batch 4096 -> 8 cores x 512 columns, identical program.
- All rotations composed on the host into block-diagonal 16x16 matrices; the
  in-rotation for non-activated rows is folded THROUGH the out-rotation into a
  single 128x128 matrix C per tile (y_nonact never materializes). Input scaling
  is folded into the first-consumer matrix columns.
- Canonical data rows live in a per-core DRAM arena [5120, 512] fp16 (row-major,
  1KB rows). Per module: one indirect-DMA gather of 1024 rows -> SBUF x-tiles,
  matmuls (fp16 weights, fp32 PSUM), smoothed-ReLU on ACT+DVE, z rows
  (live-only) indirect-scattered back, activation rows stored contiguously.
- Output = module 7's activation rows, cast fp16->fp32 on the final DMA.
"""
import numpy as np

# ---- problem constants (hardcoded per contract) ----
COLB = 16
IN_W = 1024
OUT_W = 512
DEPTH = 8
IN_L = 4
OUT_L = 4
ACT = 8
BLOCKS = 64
CURV = 1.0
GROW = BLOCKS * ACT
TOTAL = IN_W + DEPTH * GROW  # 5120
BATCH = 4096
N_CORES = 8
BL = BATCH // N_CORES  # 512
W = BLOCKS * COLB  # 1024
NTILES = 8
NBANKS = 4

LAST_EXEC_NS = None  # set when profiling enabled


# ---------------------------------------------------------------- host math
def _rotate(x, ang, stride):
    W_, B = x.shape
    xr = x.reshape(W_ // (2 * stride), 2, stride, B)
    a = ang.reshape(W_ // (2 * stride), stride)[:, :, None]
    cth, sth = np.cos(a), np.sin(a)
    lo, hi = xr[:, 0], xr[:, 1]
    return np.stack([cth * lo + sth * hi, -sth * lo + cth * hi], axis=1).reshape(W_, B)


def _module_rot_matrices(ang):
    I = np.eye(W)
    Min = I.copy()
    for l in range(IN_L):
        Min = _rotate(Min, ang[l], 2 ** (l % 4))
    Mout = I.copy()
    for l in range(OUT_L):
        Mout = _rotate(Mout, ang[IN_L + l], 2 ** ((IN_L + l) % 4))
    Min_b = np.stack([Min[16 * b:16 * b + 16, 16 * b:16 * b + 16] for b in range(BLOCKS)])
    Mout_b = np.stack([Mout[16 * b:16 * b + 16, 16 * b:16 * b + 16] for b in range(BLOCKS)])
    return Min_b, Mout_b


class _Consts:
    pass


def _build_constants(angles, biases, indices_in, scales):
    angles = np.asarray(angles, np.float64)
    biases = np.asarray(biases, np.float64)
    scales = np.asarray(scales, np.float64)
    idx = np.asarray(indices_in, np.int64)

    c = _Consts()
    c.Min, c.Mout = [], []
    for j in range(DEPTH):
        Min_b, Mout_b = _module_rot_matrices(angles[j])
        c.Min.append(Min_b)
        c.Mout.append(Mout_b)

    read_by = [set(idx[i].tolist()) for i in range(DEPTH)]
    c.z_live = []
    for j in range(DEPTH):
        if j == DEPTH - 1:
            c.z_live.append(np.zeros(W, bool))
            continue
        live = np.zeros(W, bool)
        for r in range(W):
            row = idx[j][r]
            for i in range(j + 1, DEPTH):
                if row in read_by[i]:
                    live[r] = True
                    break
        c.z_live.append(live)

    c.z_order, c.z_nlive = [], []
    for j in range(DEPTH):
        orders, nlives = [], []
        for T in range(NTILES):
            loc = np.arange(128)
            lv = c.z_live[j][128 * T + loc]
            orders.append(np.concatenate([loc[lv], loc[~lv]]))
            nlives.append(int(lv.sum()))
        c.z_order.append(orders)
        c.z_nlive.append(nlives)

    first_read = {}
    for j in range(DEPTH):
        for row in idx[j]:
            r = int(row)
            if r not in first_read:
                first_read[r] = j

    def src_factor(row, j):
        return scales[row] if (row < IN_W and first_read.get(int(row)) == j) else 1.0

    # factor per (module, tile, k): vectorized fold
    fac = np.ones((DEPTH, NTILES, 128))
    for j in range(DEPTH):
        for T in range(NTILES):
            for k in range(128):
                fac[j, T, k] = src_factor(idx[j][128 * T + k], j)

    c.W_act = np.zeros((DEPTH, NBANKS, 2, 128, 64))
    for j in range(DEPTH):
        for a in range(NBANKS):
            for h in range(2):
                T = 2 * a + h
                for m in range(64):
                    r_act = 128 * a + 64 * h + m
                    b = r_act // ACT
                    pos = r_act % ACT
                    bloc = b - 8 * T
                    ks = 16 * bloc + np.arange(16)
                    c.W_act[j, a, h, ks, m] = c.Min[j][b][pos, :] * fac[j, T, ks]

    c.C = np.zeros((DEPTH, NTILES, 128, 128))
    c.D = np.zeros((DEPTH, NTILES, 64, 128))
    for j in range(DEPTH - 1):
        Min_b, Mout_b = c.Min[j], c.Mout[j]
        # per block: composed nonact transform [16 out, 16 in]
        comp = np.einsum("bpk,bki->bpi", Mout_b[:, :, ACT:], Min_b[:, ACT:, :])
        for T in range(NTILES):
            order = c.z_order[j][T]
            for m_idx in range(128):
                r = 128 * T + order[m_idx]
                b = r // 16
                pos = r % 16
                bloc = b - 8 * T
                ks = 16 * bloc + np.arange(16)
                c.C[j, T, ks, m_idx] = comp[b][pos, :] * fac[j, T, ks]
                c.D[j, T, 8 * bloc + np.arange(ACT), m_idx] = Mout_b[b][pos, :ACT]

    c.bias = biases.reshape(DEPTH, NBANKS, 128)
    c.goff = np.zeros((DEPTH, NTILES, 128), np.int32)
    c.soff = np.zeros((DEPTH, NTILES, 128), np.int32)
    for j in range(DEPTH):
        for T in range(NTILES):
            c.goff[j, T] = idx[j][128 * T:128 * T + 128]
            c.soff[j, T] = idx[j][128 * T + c.z_order[j][T]]
    return c


# ------------------------------------------------- walrus sync-wait workaround
def _split_sync_waits(nc, limit=1):
    """This container's walrus build rejects >1 semaphore wait per instruction
    ("Too many sync wait commands"). Move excess waits onto NoOps placed just
    before the instruction on the same engine queue — the sequencer stalls at
    each NoOp's wait, so ordering semantics are identical."""
    import concourse.mybir as mybir

    seq = [0]
    for f in nc.m.functions:
        for bb in f.blocks:
            insts = bb.instructions
            newlist = []
            changed = False
            for inst in insts:
                si = getattr(inst, "sync_info", None)
                waits = list(si.on_wait) if si is not None else []
                if len(waits) > limit:
                    changed = True
                    for w in waits[:-limit]:
                        nop = mybir.InstNoOp(
                            name=f"waitsplit-{seq[0]}", ins=[], outs=[])
                        seq[0] += 1
                        nop.engine = inst.engine
                        nop.sync_info = mybir.SyncInfo(on_wait=[w], on_update=[])
                        newlist.append(nop)
                    inst.sync_info = mybir.SyncInfo(
                        on_wait=waits[-limit:], on_update=list(si.on_update))
                newlist.append(inst)
            if changed:
                bb.instructions = newlist


# ---------------------------------------------------------------- bass build
def _build_bass(c):
    import concourse.bass as bass
    import concourse.mybir as mybir
    import concourse.tile as tile
    from contextlib import ExitStack

    f16, f32, i32 = mybir.dt.float16, mybir.dt.float32, mybir.dt.int32
    AF = mybir.ActivationFunctionType
    OP = mybir.AluOpType

    nc = bass.Bass(trn_type="TRN2")
    xin = nc.dram_tensor("xin", [IN_W, BL], f32, kind="ExternalInput")
    out = nc.dram_tensor("out", [OUT_W, BL], f32, kind="ExternalOutput")

    # inline constants
    wact_np = np.zeros((128, DEPTH * NBANKS * 2 * 64), np.float16)
    for j in range(DEPTH):
        for a in range(NBANKS):
            for h in range(2):
                col = ((j * NBANKS + a) * 2 + h) * 64
                wact_np[:, col:col + 64] = c.W_act[j, a, h].astype(np.float16)
    cmat_np = np.zeros((128, (DEPTH - 1) * NTILES * 128), np.float16)
    dmat_np = np.zeros((128, (DEPTH - 1) * NTILES * 128), np.float16)
    for j in range(DEPTH - 1):
        for T in range(NTILES):
            col = (j * NTILES + T) * 128
            cmat_np[:, col:col + 128] = c.C[j, T].astype(np.float16)
            po = 64 * (T % 2)
            dmat_np[po:po + 64, col:col + 128] = c.D[j, T].astype(np.float16)
    # last column of bias_np = 0.25 constant (Sqrt bias)
    bias_np = np.zeros((128, DEPTH * NBANKS + 1), np.float32)
    bias_np[:, DEPTH * NBANKS] = 0.25
    hbias_np = np.zeros((128, DEPTH * NBANKS), np.float32)
    for j in range(DEPTH):
        for a in range(NBANKS):
            bias_np[:, j * NBANKS + a] = c.bias[j, a].astype(np.float32)
            hbias_np[:, j * NBANKS + a] = (0.5 * c.bias[j, a]).astype(np.float32)
    goff_np = np.zeros((128, DEPTH * NTILES), np.int32)
    soff_np = np.zeros((128, DEPTH * NTILES), np.int32)
    for j in range(DEPTH):
        for T in range(NTILES):
            goff_np[:, j * NTILES + T] = c.goff[j, T]
            soff_np[:, j * NTILES + T] = c.soff[j, T]

    wact_t = nc.inline_tensor(wact_np, name="wact")
    cmat_t = nc.inline_tensor(cmat_np, name="cmat")
    dmat_t = nc.inline_tensor(dmat_np, name="dmat")
    bias_t = nc.inline_tensor(bias_np, name="biast")
    hbias_t = nc.inline_tensor(hbias_np, name="hbiast")
    goff_t = nc.inline_tensor(goff_np, name="gofft")
    soff_t = nc.inline_tensor(soff_np, name="sofft")

    with tile.TileContext(nc) as tc, ExitStack() as ctx:
        const = ctx.enter_context(tc.tile_pool(name="const", bufs=1))
        xpool = ctx.enter_context(tc.tile_pool(name="x", bufs=2))
        apool = ctx.enter_context(tc.tile_pool(name="actp", bufs=2))
        zpool = ctx.enter_context(tc.tile_pool(name="z", bufs=2))
        pspool = ctx.enter_context(tc.tile_pool(name="ps", bufs=8, space="PSUM"))
        dram = ctx.enter_context(tc.tile_pool(name="dram", bufs=1, space="DRAM"))

        wact_sb = const.tile([128, wact_np.shape[1]], f16, tag="wact")
        cmat_sb = const.tile([128, cmat_np.shape[1]], f16, tag="cmat")
        dmat_sb = const.tile([128, dmat_np.shape[1]], f16, tag="dmat")
        bias_sb = const.tile([128, bias_np.shape[1]], f32, tag="bias")
        hbias_sb = const.tile([128, hbias_np.shape[1]], f32, tag="hbias")
        goff_sb = const.tile([128, goff_np.shape[1]], i32, tag="goff")
        soff_sb = const.tile([128, soff_np.shape[1]], i32, tag="soff")
        for sb_t, dr_t in [(wact_sb, wact_t), (cmat_sb, cmat_t), (dmat_sb, dmat_t),
                           (bias_sb, bias_t), (hbias_sb, hbias_t),
                           (goff_sb, goff_t), (soff_sb, soff_t)]:
            nc.sync.dma_start(out=sb_t[:], in_=dr_t[:])

        arena = dram.tile([TOTAL, BL], f16, tag="arena")

        # init: xin (f32) -> arena[0:1024] (f16) via SBUF cast
        for T in range(NTILES):
            stage32 = xpool.tile([128, BL], f32, tag="init32")
            nc.sync.dma_start(out=stage32[:], in_=xin[128 * T:128 * T + 128, :])
            stage16 = xpool.tile([128, BL], f16, tag="init16")
            nc.vector.tensor_copy(out=stage16[:], in_=stage32[:])
            nc.sync.dma_start(out=arena[128 * T:128 * T + 128, :], in_=stage16[:])

        for j in range(DEPTH):
            # ---- gather x tiles
            xs = []
            for T in range(NTILES):
                xt = xpool.tile([128, BL], f16, tag=f"x{T}")
                col = j * NTILES + T
                nc.gpsimd.indirect_dma_start(
                    out=xt[:], out_offset=None, in_=arena[:],
                    in_offset=bass.IndirectOffsetOnAxis(ap=goff_sb[:, col:col + 1], axis=0),
                )
                xs.append(xt)
            # ---- act banks + activation
            aos = []
            for a in range(NBANKS):
                ps = pspool.tile([128, BL], f32, tag="ps", space="PSUM")
                for h in range(2):
                    wcol = ((j * NBANKS + a) * 2 + h) * 64
                    nc.tensor.matmul(
                        out=ps[64 * h:64 * h + 64, :],
                        lhsT=wact_sb[:, wcol:wcol + 64],
                        rhs=xs[2 * a + h][:],
                        start=True, stop=True,
                        tile_position=(0, 64 * h),
                    )
                bcol = j * NBANKS + a
                sq = apool.tile([128, BL], f16, tag="sq")
                nc.scalar.activation(out=sq[:], in_=ps[:], func=AF.Square,
                                     bias=bias_sb[:, bcol:bcol + 1], scale=1.0)
                u = apool.tile([128, BL], f16, tag="u")
                nc.vector.tensor_scalar(out=u[:], in0=ps[:], scalar1=0.5,
                                        scalar2=hbias_sb[:, bcol:bcol + 1],
                                        op0=OP.mult, op1=OP.add)
                v = apool.tile([128, BL], f16, tag="v")
                qcol = DEPTH * NBANKS
                nc.scalar.activation(out=v[:], in_=sq[:], func=AF.Sqrt,
                                     bias=bias_sb[:, qcol:qcol + 1], scale=0.25)
                ao = apool.tile([128, BL], f16, tag=f"ao{a}")
                nc.vector.tensor_tensor(out=ao[:], in0=u[:], in1=v[:], op=OP.add)
                aos.append(ao)

            if j == DEPTH - 1:
                for a in range(NBANKS):
                    nc.gpsimd.dma_start(out=out[128 * a:128 * a + 128, :], in_=aos[a][:])
                break

            # ---- z tiles: C @ x + D @ act_out, evac live rows, scatter
            for T in range(NTILES):
                nlv = c.z_nlive[j][T]
                if nlv == 0:
                    continue
                col = (j * NTILES + T) * 128
                po = 64 * (T % 2)
                ps = pspool.tile([128, BL], f32, tag="ps", space="PSUM")
                nc.tensor.matmul(out=ps[:], lhsT=cmat_sb[:, col:col + 128],
                                 rhs=xs[T][:], start=True, stop=False)
                nc.tensor.matmul(out=ps[:], lhsT=dmat_sb[po:po + 64, col:col + 128],
                                 rhs=aos[T // 2][po:po + 64, :],
                                 start=False, stop=True, tile_position=(po, 0))
                zq = zpool.tile([128, BL], f16, tag=f"z{T}")
                if T % 2 == 0:
                    nc.vector.tensor_copy(out=zq[0:nlv, :], in_=ps[0:nlv, :])
                else:
                    nc.scalar.copy(out=zq[0:nlv, :], in_=ps[0:nlv, :])
                scol = j * NTILES + T
                nc.gpsimd.indirect_dma_start(
                    out=arena[:],
                    out_offset=bass.IndirectOffsetOnAxis(ap=soff_sb[0:nlv, scol:scol + 1], axis=0),
                    in_=zq[0:nlv, :], in_offset=None,
                )
            # ---- act rows -> arena (contiguous)
            base = IN_W + GROW * j
            for a in range(NBANKS):
                nc.sync.dma_start(out=arena[base + 128 * a: base + 128 * a + 128, :],
                                  in_=aos[a][:])
    _split_sync_waits(nc)
    return nc


# ---------------------------------------------------------------- entry point
def _time_pjrt(nc, in_maps, n_runs):
    """Replicate bass2jax.run_bass_via_pjrt's multi-core path, with a timing
    loop over executions (inputs pre-uploaded; donated zero outputs re-uploaded
    outside the timed region). Returns (results, min_wall_ns_per_exec)."""
    import time
    import jax
    import jax.numpy as jnp
    from jax.sharding import Mesh, PartitionSpec
    from jax.experimental.shard_map import shard_map
    import concourse.mybir as mybir
    from concourse import bass2jax

    bass2jax.install_neuronx_cc_hook()
    n_cores = len(in_maps)
    partition_name = nc.partition_id_tensor.name if nc.partition_id_tensor else None
    in_names, out_names, out_avals, zero_outs = [], [], [], []
    for alloc in nc.m.functions[0].allocations:
        if not isinstance(alloc, mybir.MemoryLocationSet):
            continue
        name = alloc.memorylocations[0].name
        if alloc.kind == "ExternalInput":
            if name != partition_name:
                in_names.append(name)
        elif alloc.kind == "ExternalOutput":
            shape = tuple(alloc.tensor_shape)
            dtype = mybir.dt.np(alloc.dtype)
            out_names.append(name)
            out_avals.append(jax.core.ShapedArray(shape, dtype))
            zero_outs.append(np.zeros(shape, dtype))
    n_params = len(in_names)
    n_outs = len(out_avals)
    in_names_all = in_names + out_names + ([partition_name] if partition_name else [])
    donate = tuple(range(n_params, n_params + n_outs))

    def _body(*args):
        operands = list(args)
        if partition_name is not None:
            operands.append(bass2jax.partition_id_tensor())
        outs = bass2jax._bass_exec_p.bind(
            *operands,
            out_avals=tuple(out_avals),
            in_names=tuple(in_names_all),
            out_names=tuple(out_names),
            lowering_input_output_aliases=(),
            sim_require_finite=True,
            sim_require_nnan=True,
            nc=nc,
        )
        return tuple(outs)

    devices = jax.devices()[:n_cores]
    mesh = Mesh(np.asarray(devices), ("core",))
    sharded = jax.jit(
        shard_map(_body, mesh=mesh,
                  in_specs=(PartitionSpec("core"),) * (n_params + n_outs),
                  out_specs=(PartitionSpec("core"),) * n_outs, check_rep=False),
        donate_argnums=donate, keep_unused=True,
    )
    concat_in = [
        np.concatenate([np.asarray(in_maps[c][name]) for c in range(n_cores)], axis=0)
        for name in in_names
    ]
    concat_zero_shapes = [((n_cores * z.shape[0],) + z.shape[1:], z.dtype)
                          for z in zero_outs]
    from jax.sharding import NamedSharding
    shin = NamedSharding(mesh, PartitionSpec("core"))
    dev_in = [jax.device_put(x, shin) for x in concat_in]

    best = None
    out_arrs = None
    for run in range(max(1, n_runs)):
        dev_zeros = [jax.device_put(jnp.zeros(s, d), shin) for s, d in concat_zero_shapes]
        for z in dev_zeros:
            z.block_until_ready()
        t0 = time.perf_counter()
        out_arrs = sharded(*dev_in, *dev_zeros)
        for o in out_arrs:
            o.block_until_ready()
        t1 = time.perf_counter()
        dt = (t1 - t0) * 1e9
        best = dt if best is None else min(best, dt)
    results = [
        {name: np.asarray(out_arrs[i]).reshape(n_cores, *out_avals[i].shape)[cix]
         for i, name in enumerate(out_names)}
        for cix in range(n_cores)
    ]
    return results, best


def kernel(input_data, scales, angles, biases, indices_in, _profile=False):
    global LAST_EXEC_NS
    input_data = np.ascontiguousarray(np.asarray(input_data, np.float32))
    c = _build_constants(angles, biases, indices_in, scales)
    nc = _build_bass(c)
    in_maps = [{"xin": np.ascontiguousarray(input_data[:, i * BL:(i + 1) * BL])}
               for i in range(N_CORES)]
    if _profile:
        results, best_ns = _time_pjrt(nc, in_maps, n_runs=12)
        LAST_EXEC_NS = int(best_ns)
    else:
        from concourse import bass_utils
        res = bass_utils.run_bass_kernel_spmd(
            nc, in_maps, core_ids=list(range(N_CORES)), trace=False,
        )
        results = res.results
        LAST_EXEC_NS = res.exec_time_ns
    out = np.concatenate([r["out"] for r in results], axis=1)
    return out.astype(np.float32)
